# revision 30
# baseline (speedup 1.0000x reference)
"""Cumulative-probability head on 8 Trainium2 NeuronCores.

out[b, j] = sum_{i<=j} relu(x @ W_h^T + b_h)[b, i] + (x @ W_base^T + b_base)[b]

Data-parallel: x sharded along batch (1024 rows/core), weights replicated.

Per-core strategy (fp8 DoubleRow):
  - x and W quantized host-side to TRN fp8-e4m3 (ml_dtypes.float8_e4m3,
    max normal 240) with power-of-2 scales Sx=16, Sw=512; matmuls run in
    MatmulPerfMode.DoubleRow(SwInterleave) — 2 fp8 MACs/cell/cycle —
    accumulating S*(x@W) in fp32 PSUM.
  - Contraction 2048 = 8 chunks x (128 partitions x 2 doublerow slots).
  - Hazard matmul N=512 fills one PSUM bank; the base column rides as a
    tiny N=2 matmul sharing the stationary.
  - Batch in 2 waves of 512 rows: wave 0 chunk-outer (paced by the input
    DMA stream), wave 1 b-outer (drains overlap later tiles' matmuls).
    Dummy matmuls during the initial DMA window warm the HAM clock.
  - Post per b-tile: DVE adds S*bias into PSUM in place, ScalarE Relu
    with 1/S descale into bf16, DVE tensor_tensor_scan (fp32 state) does
    the inclusive cumsum seeded with the base hazard, bf16 out DMA.
  - Input DMAs spread over Sync/Scalar HWDGE + GPSIMD SWDGE rings in
    k-order; wave-1 x streams during wave-0 compute.
"""

import numpy as np
import ml_dtypes

import concourse.bass as bass
import concourse.tile as tile
from concourse import bacc, mybir
from concourse.bass_utils import run_bass_kernel_spmd

B, D, T = 8192, 2048, 512
NCORES = 8
BLOC = B // NCORES            # 1024 rows per core
WB = BLOC // 2                # 512 rows per wave
NBW = WB // 128               # 4 b-tiles per wave
NCH = D // 256                # 8 contraction chunks (256 = 128 x 2 doublerow)
TP = 516                      # padded W width: 512 hazard + base + 3 zero
SX = 16.0                     # x fp8 scale
SW = 512.0                    # W fp8 scale
S = SX * SW
SWI = True                    # stationary uses DoubleRowSwInterleave layout

F32 = mybir.dt.float32
BF16 = mybir.dt.bfloat16
F8 = mybir.dt.float8e4

F8NP = ml_dtypes.float8_e4m3
BF16NP = ml_dtypes.bfloat16


def _build_program():
    nc = bacc.Bacc("TRN2", target_bir_lowering=False, debug=False)

    # x layout: [wave, chunk, 128 partitions, 1024B free] — for SWI the
    # free dim is (bl, m-reversed, slot)-interleaved; for plain DoubleRow
    # it is (slot, batch-col).
    xt_d = nc.dram_tensor("xt", [2, NCH, 128, 2 * WB], F8, kind="ExternalInput")
    wt_d = nc.dram_tensor("wt", [D, TP], F8, kind="ExternalInput")
    bias_d = nc.dram_tensor("bias", [1, TP], BF16, kind="ExternalInput")
    bias8_d = nc.dram_tensor("bias8", [1, TP], F8, kind="ExternalInput")
    out_d = nc.dram_tensor("out", [BLOC, T], BF16, kind="ExternalOutput")

    DR = (
        mybir.MatmulPerfMode.DoubleRowSwInterleave
        if SWI
        else mybir.MatmulPerfMode.DoubleRow
    )
    Relu = mybir.ActivationFunctionType.Relu
    Ident = mybir.ActivationFunctionType.Identity

    with tile.TileContext(nc) as tc:
        with (
            tc.tile_pool(name="consts", bufs=1) as consts,
            tc.tile_pool(name="wt", bufs=1) as wtp,
            tc.tile_pool(name="xt", bufs=1) as xtp,
            tc.tile_pool(name="haz", bufs=4) as hazp,
            tc.tile_pool(name="outp", bufs=4) as outp,
            tc.tile_pool(name="ps", bufs=5, space="PSUM") as psp,
            tc.tile_pool(name="psb", bufs=1, space="PSUM") as psbp,
            tc.tile_pool(name="psb1", bufs=2, space="PSUM") as psb1p,
        ):
            zeros = consts.tile([128, T], BF16, tag="zeros")
            nc.vector.memset(zeros, 0.0)
            bias_bc = consts.tile([128, TP], BF16, tag="bias")
            # Wave-1 tiles take the hazard bias on the PE instead of the
            # DVE: a closer matmul (stop=True) of a constant stationary
            # (one 16.0 cell) times a fp8 row of (S/16)*b_hazard. It runs
            # late in each group, so its inputs are never on the
            # critical path, and it unloads the DVE in the window where
            # drain work otherwise overhangs past the last matmul.
            ones_t = consts.tile([128, 2, 128], F8, tag="ones")
            nc.vector.memset(ones_t, 0.0)
            nc.vector.memset(ones_t[0:1, 0, :], 16.0)
            wt8 = consts.tile([128, 2, TP], F8, tag="wt8")
            nc.vector.memset(wt8, 0.0)

            wt_tiles = [
                wtp.tile([128, 2, TP], F8, tag=f"wt{c}", name=f"wt{c}")
                for c in range(NCH)
            ]
            xt_tiles = [
                [
                    xtp.tile([128, 2 * WB], F8, tag=f"x{w}_{c}", name=f"x{w}_{c}")
                    for c in range(NCH)
                ]
                for w in range(2)
            ]
            # Explicit per-ring DMA queues (FIFO per ring), ordered so chunk
            # c's pair lands just ahead of the PE's ~0.95us/chunk cadence.
            bsrc = bias_d[0:1, :]
            bias_bc_ap = bass.AP(
                tensor=bsrc.tensor,
                offset=bsrc.offset,
                ap=[[0, 128]] + list(bsrc.ap[1:]),
            )
            w0, x0, x1 = wt_tiles, xt_tiles[0], xt_tiles[1]
            sync_q = [
                (w0[0][:, :, 0:258], wt_d[0:256, 0:258]),
                (x0[0][:, WB : 2 * WB], xt_d[0, 0, :, WB : 2 * WB]),
                (x0[1], xt_d[0, 1]),
                (x0[4], xt_d[0, 4]),
                (w0[5], wt_d[1280:1536, :]),
                (x1[0], xt_d[1, 0]),
                (x1[3], xt_d[1, 3]),
                (x1[6], xt_d[1, 6]),
            ]
            scalar_q = [
                (w0[0][:, :, 258:TP], wt_d[0:256, 258:TP]),
                (w0[1], wt_d[256:512, :]),
                (w0[2], wt_d[512:768, :]),
                (x0[3], xt_d[0, 3]),
                (w0[4], wt_d[1024:1280, :]),
                (x0[6], xt_d[0, 6]),
                (w0[7], wt_d[1792:2048, :]),
                (x1[1], xt_d[1, 1]),
                (x1[4], xt_d[1, 4]),
                (x1[7], xt_d[1, 7]),
            ]
            gpsimd_q = [
                (x0[0][:, 0:WB], xt_d[0, 0, :, 0:WB]),
                (x0[2], xt_d[0, 2]),
                (w0[3], wt_d[768:1024, :]),
                (x0[5], xt_d[0, 5]),
                (bias_bc, bias_bc_ap),
                (wt8[0:1, 0, :], bias8_d[0:1, :]),
                (w0[6], wt_d[1536:1792, :]),
                (x0[7], xt_d[0, 7]),
                (x1[2], xt_d[1, 2]),
                (x1[5], xt_d[1, 5]),
            ]
            for ring, q in ((nc.sync, sync_q), (nc.scalar, scalar_q), (nc.gpsimd, gpsimd_q)):
                for dst, src in q:
                    ring.dma_start(out=dst, in_=src)

            base_ps0 = psbp.tile([128, 2 * NBW], F32, tag="bps")
            out_rings = [nc.scalar, nc.sync]

            def lhsT_of(wv, c, bl):
                xk = xt_tiles[wv][c]
                if SWI:
                    # Stationary stored interleaved: within the 256-byte
                    # b-tile block, element q = 2*(127-m) + i. AP dims
                    # [partition, pair(stride 1), m(stride 2)].
                    sl = xk[:, 256 * bl : 256 * (bl + 1)]
                    return bass.AP(
                        tensor=sl.tensor,
                        offset=sl.offset,
                        ap=[list(sl.ap[0]), [1, 2], [2, 128]],
                    )
                # Plain DoubleRow: [partition, slot(stride WB), m(stride 1)].
                sl = xk[:, 128 * bl : 128 * (bl + 1)]
                return bass.AP(
                    tensor=sl.tensor,
                    offset=sl.offset,
                    ap=[list(sl.ap[0]), [WB, 2], [1, 128]],
                )

            def mm_pair(ps, bps, wv, c, bl, haz_stop=True):
                lhsT = lhsT_of(wv, c, bl)
                nc.tensor.matmul(
                    ps[:],
                    lhsT,
                    wt_tiles[c][:, :, 0:T],
                    start=(c == 0),
                    stop=(c == NCH - 1) and haz_stop,
                    perf_mode=DR,
                )
                nc.tensor.matmul(
                    bps,
                    lhsT,
                    wt_tiles[c][:, :, T : T + 2],
                    start=(c == 0),
                    stop=(c == NCH - 1),
                    perf_mode=DR,
                )

            def bias_close_mm(ps):
                # Adds S*b_hazard into the accumulation and closes the group.
                nc.tensor.matmul(
                    ps[:],
                    ones_t[:],
                    wt8[:, :, 0:T],
                    start=False,
                    stop=True,
                    perf_mode=mybir.MatmulPerfMode.DoubleRow,
                )

            def drain(ps, bps, b, nsplit=1, dve_bias=True):
                # psum += S*bias (wave 0: in place on DVE; wave 1: already
                # added by the PE closer), Relu with descale into bf16, then
                # the inclusive scan seeded with the base hazard; nsplit>1
                # chains scan segments so segment DMAs overlap later scans
                # (used on the last tile to shorten the kernel tail).
                if dve_bias:
                    nc.vector.tensor_add(ps[:], ps[:], bias_bc[:, 0:T])
                haz = hazp.tile([128, T], BF16, tag="haz", name=f"haz{b}")
                nc.scalar.activation(out=haz, in_=ps[:], func=Relu, scale=1.0 / S)
                baset = hazp.tile([128, 1], BF16, tag="base", name=f"base{b}")
                nc.scalar.activation(
                    out=baset,
                    in_=bps[:, 0:1],
                    func=Ident,
                    scale=1.0 / S,
                    bias=bias_bc[:, T : T + 1],
                )
                cum = outp.tile([128, T], BF16, tag="cum", name=f"cum{b}")
                H = T // nsplit
                for q in range(nsplit):
                    lo, hi = q * H, (q + 1) * H
                    nc.vector.tensor_tensor_scan(
                        out=cum[:, lo:hi],
                        data0=haz[:, lo:hi],
                        data1=zeros[:, lo:hi],
                        initial=baset if q == 0 else cum[:, lo - 1 : lo],
                        op0=mybir.AluOpType.add,
                        op1=mybir.AluOpType.add,
                    )
                    out_rings[b % 2].dma_start(
                        out=out_d[128 * b : 128 * (b + 1), lo:hi], in_=cum[:, lo:hi]
                    )

            # Wave 0 (b-tiles 0..3): chunk-outer so the PE paces with the
            # incoming chunk stream; drains overlap wave-1 compute.
            ps0 = [
                psp.tile([128, T], F32, tag="ps", name=f"ps_0_{i}")
                for i in range(NBW)
            ]
            # HAM warm-up: the PE clock runs at 4/8 until ~3.4us of
            # continuous activity. Burn that in on dummy bf16 matmuls
            # (zeros x zeros into ps0[0], overwritten by the real wave-0
            # start=True) during the otherwise-idle DMA-wait window.
            for i in range(6):
                nc.tensor.matmul(
                    ps0[0][:],
                    zeros[:, 0:128],
                    zeros[:, 0:T],
                    start=True,
                    stop=True,
                )
            for c in range(NCH):
                for bl in range(NBW):
                    mm_pair(ps0[bl], base_ps0[:, 2 * bl : 2 * bl + 2], 0, c, bl)
            for bl in range(NBW):
                drain(ps0[bl], base_ps0[:, 2 * bl : 2 * bl + 2], bl)

            # Wave 1 (b-tiles 4..7): inputs are resident, go b-outer — each
            # tile's accumulation stops early and its drain overlaps the
            # next tile's matmuls. Per-b base PSUM tiles so a drain's read
            # doesn't block the next tile's base matmul.
            for bl in range(NBW):
                b = NBW + bl
                # PE bias-closers only for the last two tiles, where the
                # DVE drain queue would otherwise overhang past the last
                # matmul; the earlier tiles' DVE adds run in a free window.
                close_on_pe = bl >= NBW - 2
                ps = psp.tile([128, T], F32, tag="ps", name=f"ps_1_{bl}")
                bps = psb1p.tile([128, 2], F32, tag="bps1", name=f"bps_1_{bl}")
                for c in range(NCH):
                    mm_pair(ps, bps, 1, c, bl, haz_stop=not close_on_pe)
                if close_on_pe:
                    bias_close_mm(ps)
                drain(
                    ps,
                    bps,
                    b,
                    nsplit=2 if bl == NBW - 1 else 1,
                    dve_bias=not close_on_pe,
                )

    nc.compile()
    return nc


_NC_CACHE = None


def prep_in_maps(x, W_hazard, b_hazard, W_base, b_base):
    x = np.asarray(x, np.float32)
    Wh = np.asarray(W_hazard, np.float32)
    bh = np.asarray(b_hazard, np.float32)
    Wb = np.asarray(W_base, np.float32).reshape(1, D)
    bb = np.asarray(b_base, np.float32).reshape(1)

    wt = np.zeros((D, TP), np.float32)
    wt[:, 0 : T + 1] = np.concatenate([Wh, Wb], axis=0).T * SW
    np.clip(wt, -240.0, 240.0, out=wt)
    wt8 = wt.astype(F8NP)

    bias = np.zeros((1, TP), np.float32)
    bias[0, 0:T] = bh * S
    bias[0, T] = bb[0]
    bias16 = bias.astype(BF16NP)
    # fp8 hazard-bias row for the wave-1 PE closer matmul: the stationary
    # cell is 16.0, so the row carries (S/16)*b to contribute exactly S*b.
    bias8 = np.zeros((1, TP), np.float32)
    bias8[0, 0:T] = bh * (S / 16.0)
    np.clip(bias8, -240.0, 240.0, out=bias8)
    bias8 = bias8.astype(F8NP)

    x8 = np.clip(x * SX, -240.0, 240.0).astype(F8NP)  # [B, D]
    in_maps = []
    for i in range(NCORES):
        xs = x8[BLOC * i : BLOC * (i + 1)]  # [1024, D]
        # [w, bl, m, c, p, i] view of the shard
        Y = xs.reshape(2, NBW, 128, NCH, 128, 2)
        if SWI:
            # Stationary interleave: block q = 2*(127-m) + i.
            Y = Y[:, :, ::-1, :, :, :]
            xt = np.ascontiguousarray(
                Y.transpose(0, 3, 4, 1, 2, 5).reshape(2, NCH, 128, 2 * WB)
            )
        else:
            # Plain DoubleRow: free dim is (slot i, batch col).
            xt = np.ascontiguousarray(
                Y.transpose(0, 3, 4, 5, 1, 2).reshape(2, NCH, 128, 2 * WB)
            )
        in_maps.append({"xt": xt, "wt": wt8, "bias": bias16, "bias8": bias8})
    return in_maps


def kernel(x, W_hazard, b_hazard, W_base, b_base):
    global _NC_CACHE
    if _NC_CACHE is None:
        _NC_CACHE = _build_program()
    in_maps = prep_in_maps(x, W_hazard, b_hazard, W_base, b_base)
    res = run_bass_kernel_spmd(_NC_CACHE, in_maps, list(range(NCORES)))
    return np.concatenate(
        [res.results[i]["out"].astype(np.float32) for i in range(NCORES)], axis=0
    )
